# revision 28
# baseline (speedup 1.0000x reference)
"""NTM read controller kernel for Trainium2 (8 NeuronCores, SPMD data-parallel over batch).

Problem shapes (hardcoded): B=64, E=512, N=8192, M=64, Dense out = M+6 = 70.
Sharding: batch 64 -> 8 cores x 8 batches. Each core is fully independent.

Per-core layout: n = 64*p + j  (partition p in [0,128), j in [0,64)).
memory_weights[b] loads as a [128, 4096] SBUF slab with 16 KiB contiguous per
partition (full-rate DMA).  Engine split:
  - DVE: dot-product mul + grouped reduces + small chain ops
  - ACT: square pass, Exp/Ln/Sqrt/Softplus/Sigmoid (scale = per-partition AP)
  - PE : controller matmul, partition-sum broadcast (all-ones matmul),
         circular-shift boundary columns (permutation matmul), weighted read
"""

import sys

for _p in ("/opt/trn_rl_repo", "/root/.axon_site/_ro/trn_rl_repo"):
    if _p not in sys.path:
        sys.path.insert(0, _p)

import numpy as np

import concourse.bass as bass
import concourse.bacc as bacc
import concourse.mybir as mybir
from concourse.tile import TileContext

F32 = mybir.dt.float32
F32R = mybir.dt.float32r
BF16 = mybir.dt.bfloat16
AF = mybir.ActivationFunctionType
ALU = mybir.AluOpType
AX = mybir.AxisListType

B_LOC = 8      # batches per core
E = 512
N = 8192
M = 64
C_OUT = 70     # M + 6
P = 128        # partitions
J = 64         # n = 64*p + j
EPS = 1e-8

_NC_CACHE = None


def _build_nc():
    nc = bacc.Bacc("TRN2", target_bir_lowering=False, debug=False, num_devices=8)

    emb_d = nc.dram_tensor("embeddings", [B_LOC, E], F32, kind="ExternalInput")
    wp_d = nc.dram_tensor("w_prev", [B_LOC, N], F32, kind="ExternalInput")
    mem_d = nc.dram_tensor("memory_weights", [B_LOC, N, M], F32, kind="ExternalInput")
    W_d = nc.dram_tensor("W", [E, C_OUT], F32, kind="ExternalInput")
    b_d = nc.dram_tensor("b", [C_OUT], F32, kind="ExternalInput")
    md_d = nc.dram_tensor("memory_data", [B_LOC, M], F32, kind="ExternalOutput")
    wout_d = nc.dram_tensor("w_out", [B_LOC, N], F32, kind="ExternalOutput")

    with TileContext(nc) as tc:
        with (
            tc.tile_pool(name="const", bufs=1) as cpool,
            tc.tile_pool(name="slab", bufs=2) as spool,
            tc.tile_pool(name="small", bufs=3) as smpool,
            tc.tile_pool(name="pscratch", bufs=2, space="PSUM") as ppool,
            tc.tile_pool(name="pmd", bufs=2, space="PSUM") as pmd,
        ):
            # ---------------- constants ----------------
            # Pre-load the one ACT table set covering every function this
            # kernel uses (ln/exp/square/copy) so the table-load pass never
            # has to switch sets inside the loop.
            _ltl = mybir.InstLoadActFuncSet(
                name=nc.get_next_instruction_name(), ins=[], outs=[]
            )
            _ltl.act_func_set_id = 6  # natural_log_exp_and_others
            nc.scalar.add_instruction(_ltl)

            ones128 = cpool.tile([P, P], F32)
            nc.vector.memset(ones128, 1.0)

            id8 = cpool.tile([8, 8], F32)
            nc.vector.memset(id8, 1.0)
            nc.gpsimd.affine_select(
                id8, id8, pattern=[[-1, 8]], compare_op=ALU.is_equal,
                fill=0.0, base=0, channel_multiplier=1,
            )

            # roll(+1): out[p] = in[(p-1) mod 128]  -> Sdn[p, j]=1 iff j=p+1, patch [127,0]
            sdn = cpool.tile([P, P], F32)
            nc.vector.memset(sdn, 1.0)
            nc.gpsimd.affine_select(
                sdn, sdn, pattern=[[-1, P]], compare_op=ALU.is_equal,
                fill=0.0, base=1, channel_multiplier=1,
            )
            # wrap element (127, 0) via a second mask + add (engine APs must
            # start at partition 0/32/64/96, so no direct offset memset)
            sdn_wrap = cpool.tile([P, P], F32)
            nc.vector.memset(sdn_wrap, 1.0)
            nc.gpsimd.affine_select(
                sdn_wrap, sdn_wrap, pattern=[[-1, P]], compare_op=ALU.is_equal,
                fill=0.0, base=-(P - 1), channel_multiplier=1,
            )
            nc.vector.tensor_tensor(sdn, sdn, sdn_wrap, ALU.add)

            # roll(-1): out[p] = in[(p+1) mod 128]  -> Sup[p, j]=1 iff j=p-1, patch [0,127]
            sup = cpool.tile([P, P], F32)
            nc.vector.memset(sup, 1.0)
            nc.gpsimd.affine_select(
                sup, sup, pattern=[[-1, P]], compare_op=ALU.is_equal,
                fill=0.0, base=-1, channel_multiplier=1,
            )
            nc.vector.memset(sup[0:1, P - 1 : P], 1.0)

            ones18 = cpool.tile([1, 8], F32)
            nc.vector.memset(ones18, 1.0)

            # ---------------- small inputs ----------------
            W4 = cpool.tile([P, 4 * C_OUT], F32)
            nc.sync.dma_start(
                out=W4.rearrange("p (c j) -> p c j", c=4),
                in_=W_d.rearrange("(c p) j -> p c j", p=P),
            )
            b_sb = cpool.tile([1, C_OUT], F32)
            nc.sync.dma_start(out=b_sb, in_=b_d.rearrange("(a j) -> a j", a=1))

            emb_sb = cpool.tile([B_LOC, E], F32)
            nc.sync.dma_start(out=emb_sb, in_=emb_d[:, :])

            wp_slab = cpool.tile([P, B_LOC * J], F32)
            nc.sync.dma_start(
                out=wp_slab.rearrange("p (b j) -> p b j", b=B_LOC),
                in_=wp_d.rearrange("b (p j) -> p b j", p=P),
            )

            w_slab = cpool.tile([P, B_LOC * J], F32)   # final w output staging
            md_row = cpool.tile([1, B_LOC * M], F32)   # final memory_data staging
            md_gather = cpool.tile([1, B_LOC * 8 * M], F32)  # diag blocks staging

            # ---------------- controller: addr = emb @ W + b ----------------
            eT_sb = cpool.tile([P, 32], F32)  # 4 chunks of emb^T [128, 8]
            for c in range(4):
                eT_ps = ppool.tile([P, 8], F32, name=f"eT_ps{c}", tag="scratch")
                nc.tensor.matmul(
                    eT_ps, emb_sb[:, c * P : (c + 1) * P], id8, is_transpose=True
                )
                nc.vector.tensor_copy(eT_sb[:, c * 8 : (c + 1) * 8], eT_ps)

            addr_ps = ppool.tile([B_LOC, C_OUT], F32, tag="scratch")
            for c in range(4):
                nc.tensor.matmul(
                    addr_ps,
                    eT_sb[:, c * 8 : (c + 1) * 8],
                    W4.rearrange("p (c j) -> p c j", c=4)[:, c, :],
                    start=(c == 0),
                    stop=False,
                )
            nc.tensor.matmul(addr_ps, ones18, b_sb, start=False, stop=True)
            addr_sb = cpool.tile([B_LOC, C_OUT], F32)
            nc.vector.tensor_copy(addr_sb, addr_ps)

            # ---------------- controller nonlinearities ----------------
            # ctrl columns: 0=beta 1=g 2=1-g 3=y 4=k_norm 5..7=s
            ctrl = cpool.tile([B_LOC, 8], F32)
            sp_t = cpool.tile([B_LOC, 2], F32)  # softplus scratch: [beta_raw, y_raw]
            nc.scalar.activation(sp_t[:, 0:1], addr_sb[:, 64:65], AF.Exp)
            nc.scalar.activation(sp_t[:, 1:2], addr_sb[:, 69:70], AF.Exp)
            nc.vector.tensor_scalar(sp_t, sp_t, 1.0, None, ALU.add)
            nc.scalar.activation(ctrl[:, 0:1], sp_t[:, 0:1], AF.Ln)       # beta
            nc.scalar.activation(ctrl[:, 3:4], sp_t[:, 1:2], AF.Ln)      # softplus(y_raw)
            # sigmoid via exp to stay inside the ln/exp table set:
            # g = 1/(1+exp(-x))
            emg = cpool.tile([B_LOC, 1], F32)
            nc.scalar.activation(emg, addr_sb[:, 65:66], AF.Exp, scale=-1.0)
            nc.vector.tensor_scalar(emg, emg, 1.0, None, ALU.add)
            nc.vector.reciprocal(ctrl[:, 1:2], emg)
            nc.vector.tensor_scalar(
                ctrl[:, 2:3], ctrl[:, 1:2], -1.0, 1.0, ALU.mult, ALU.add
            )
            nc.vector.tensor_scalar(ctrl[:, 3:4], ctrl[:, 3:4], 1.0, None, ALU.add)

            ksq = cpool.tile([B_LOC, M], F32)
            knsq = cpool.tile([B_LOC, 1], F32)
            nc.scalar.activation(ksq, addr_sb[:, 0:M], AF.Square, accum_out=knsq)
            # sqrt(x) = exp(0.5*ln(x)) — stays in the exp/ln ACT table set
            knl = cpool.tile([B_LOC, 1], F32)
            nc.scalar.activation(knl, knsq, AF.Ln)
            nc.scalar.activation(ctrl[:, 4:5], knl, AF.Exp, scale=0.5)

            sexp = cpool.tile([B_LOC, 3], F32)
            ssum = cpool.tile([B_LOC, 1], F32)
            nc.scalar.activation(sexp, addr_sb[:, 66:69], AF.Exp, accum_out=ssum)
            srec = cpool.tile([B_LOC, 1], F32)
            nc.vector.reciprocal(srec, ssum)
            nc.vector.tensor_scalar_mul(ctrl[:, 5:8], sexp, srec)

            # ---------------- per-batch broadcast of controller scalars ----------------
            cb_list = []
            kb_list = []
            ones8p = cpool.tile([B_LOC, P], F32)
            nc.vector.memset(ones8p, 1.0)
            for b in range(B_LOC):
                # sel[p, :] = 1 iff p == b  (row-select mask, built from partition 0)
                sel = cpool.tile([B_LOC, P], F32, name=f"sel{b}")
                nc.gpsimd.affine_select(
                    sel, ones8p, pattern=[[0, P]], compare_op=ALU.is_equal,
                    fill=0.0, base=-b, channel_multiplier=1,
                )

                cb_ps = ppool.tile([P, 8], F32, name=f"cb_ps{b}", tag="scratch")
                nc.tensor.matmul(cb_ps, sel, ctrl)
                cb = cpool.tile([P, 8], F32, name=f"cb{b}")
                nc.vector.tensor_copy(cb, cb_ps)
                cb_list.append(cb)

                kb_ps = ppool.tile([P, M], F32, name=f"kb_ps{b}", tag="scratch")
                nc.tensor.matmul(kb_ps, sel, addr_sb[:, 0:M])
                kb = cpool.tile([P, M], F32, name=f"kb{b}")
                nc.vector.tensor_copy(kb, kb_ps)
                kb_list.append(kb)

            # ---------------- main per-batch loop ----------------
            for b in range(B_LOC):
                cb = cb_list[b]
                beta_bc = cb[:, 0:1]
                g_bc = cb[:, 1:2]
                omg_bc = cb[:, 2:3]
                y_bc = cb[:, 3:4]
                kn_bc = cb[:, 4:5]
                s0_bc = cb[:, 5:6]
                s1_bc = cb[:, 6:7]
                s2_bc = cb[:, 7:8]
                kb = kb_list[b]

                mem_sb = spool.tile([P, J * M], F32, name=f"mem{b}", tag="mem")
                nc.sync.dma_start(
                    out=mem_sb,
                    in_=mem_d[b].rearrange("(p j) m -> p (j m)", p=P),
                )
                mem3 = mem_sb.rearrange("p (j m) -> p j m", m=M)

                # dot[p, j] = sum_m mem[p, j, m] * k[m]
                prod = spool.tile([P, J * M], F32, name=f"prod{b}", tag="prod")
                kb3 = kb.unsqueeze(1).broadcast_to([P, J, M])
                # big elementwise mul on GpSimd — the only otherwise-idle engine
                nc.gpsimd.tensor_tensor(
                    prod.rearrange("p (j m) -> p j m", m=M), mem3, kb3, ALU.mult
                )
                dot = smpool.tile([P, J], F32, name=f"dot{b}", tag="dot")
                nc.vector.tensor_reduce(
                    dot, prod.rearrange("p (j m) -> p j m", m=M), AX.X, ALU.add
                )

                # bf16 copy of mem for the fast read matmuls (1 cyc/row on PE)
                memb = spool.tile([P, J * M], BF16, name=f"memb{b}", tag="memb")
                nc.scalar.activation(memb, mem_sb, AF.Copy)

                # normsq[p, j] = sum_m mem^2
                sq = spool.tile([P, J * M], F32, name=f"sq{b}", tag="sq")
                nc.scalar.activation(sq, mem_sb, AF.Square)
                nsq = smpool.tile([P, J], F32, name=f"nsq{b}", tag="nsq")
                nc.vector.tensor_reduce(
                    nsq, sq.rearrange("p (j m) -> p j m", m=M), AX.X, ALU.add
                )

                # sim = dot / (sqrt(nsq) * k_norm + EPS); sqrt via exp(0.5*ln)
                lnn = smpool.tile([P, J], F32, name=f"lnn{b}", tag="lnn")
                nc.scalar.activation(lnn, nsq, AF.Ln)
                mnorm = smpool.tile([P, J], F32, name=f"mnorm{b}", tag="mnorm")
                nc.scalar.activation(mnorm, lnn, AF.Exp, scale=0.5)
                den = smpool.tile([P, J], F32, name=f"den{b}", tag="den")
                nc.scalar.activation(den, mnorm, AF.Copy, bias=EPS, scale=kn_bc)
                rden = smpool.tile([P, J], F32, name=f"rden{b}", tag="rden")
                nc.vector.reciprocal(rden, den)
                sim = smpool.tile([P, J], F32, name=f"sim{b}", tag="sim")
                nc.vector.tensor_tensor(sim, dot, rden, ALU.mult)

                # w_c = softmax(beta * sim) over all n (no max-sub: |beta*sim| small)
                e_t = smpool.tile([P, J], F32, name=f"e{b}", tag="e")
                esum = smpool.tile([P, 1], F32, name=f"esum{b}", tag="esum")
                nc.scalar.activation(e_t, sim, AF.Exp, scale=beta_bc, accum_out=esum)
                z_ps = ppool.tile([P, 1], F32, name=f"z_ps{b}", tag="scratch")
                nc.tensor.matmul(z_ps, ones128, esum)
                zr = smpool.tile([P, 1], F32, name=f"zr{b}", tag="zr")
                nc.vector.reciprocal(zr, z_ps)
                # fold softmax normalization and g together: t1 = e * (g/Z)
                gzr = smpool.tile([P, 1], F32, name=f"gzr{b}", tag="gzr")
                nc.vector.tensor_scalar_mul(gzr, zr, g_bc)

                # w_g = (e*(g/Z)) + (1-g)*w_prev  — one STT + one ACT copy
                t2 = smpool.tile([P, J], F32, name=f"t2{b}", tag="t2")
                nc.scalar.activation(
                    t2, wp_slab[:, b * J : (b + 1) * J], AF.Copy, scale=omg_bc
                )
                wg = smpool.tile([P, J], F32, name=f"wg{b}", tag="wg")
                nc.vector.scalar_tensor_tensor(wg, e_t, gzr, t2, ALU.mult, ALU.add)

                # circular rolls along n = 64p + j
                r1 = smpool.tile([P, J], F32, name=f"r1{b}", tag="r1")
                nc.vector.tensor_copy(r1[:, 1:J], wg[:, 0 : J - 1])
                c1_ps = ppool.tile([P, 1], F32, name=f"c1_ps{b}", tag="scratch")
                nc.tensor.matmul(c1_ps, sdn, wg[:, J - 1 : J])
                nc.vector.tensor_copy(r1[:, 0:1], c1_ps)

                rm1 = smpool.tile([P, J], F32, name=f"rm1{b}", tag="rm1")
                nc.vector.tensor_copy(rm1[:, 0 : J - 1], wg[:, 1:J])
                cm_ps = ppool.tile([P, 1], F32, name=f"cm_ps{b}", tag="scratch")
                nc.tensor.matmul(cm_ps, sup, wg[:, 0:1])
                nc.vector.tensor_copy(rm1[:, J - 1 : J], cm_ps)

                # w_s = s0*r1 + s1*wg + s2*rm1  (ACT mul + two fused STTs)
                a2 = smpool.tile([P, J], F32, name=f"a2{b}", tag="a2")
                nc.scalar.activation(a2, wg, AF.Copy, scale=s1_bc)
                t12 = smpool.tile([P, J], F32, name=f"t12{b}", tag="t12")
                nc.vector.scalar_tensor_tensor(t12, r1, s0_bc, a2, ALU.mult, ALU.add)
                ws = smpool.tile([P, J], F32, name=f"ws{b}", tag="ws")
                nc.vector.scalar_tensor_tensor(ws, rm1, s2_bc, t12, ALU.mult, ALU.add)

                # w = ws^y / (sum + EPS)
                lnw = smpool.tile([P, J], F32, name=f"lnw{b}", tag="lnw")
                nc.scalar.activation(lnw, ws, AF.Ln)
                wpw = smpool.tile([P, J], F32, name=f"wpw{b}", tag="wpw")
                wps = smpool.tile([P, 1], F32, name=f"wps{b}", tag="wps")
                nc.scalar.activation(wpw, lnw, AF.Exp, scale=y_bc, accum_out=wps)
                zp_ps = ppool.tile([P, 1], F32, name=f"zp_ps{b}", tag="scratch")
                nc.tensor.matmul(zp_ps, ones128, wps)
                zpe = smpool.tile([P, 1], F32, name=f"zpe{b}", tag="zpe")
                nc.vector.tensor_scalar(zp_ps_sb := zpe, zp_ps, EPS, None, ALU.add)
                zpr = smpool.tile([P, 1], F32, name=f"zpr{b}", tag="zpr")
                nc.vector.reciprocal(zpr, zp_ps_sb)
                nc.scalar.activation(
                    w_slab[:, b * J : (b + 1) * J], wpw, AF.Copy, scale=zpr
                )

                # read: memory_data[b, m] = sum_n w[n] mem[n, m]
                # quad-j float32r matmuls: lhsT = 4 w columns, rhs = 4 j-groups
                # (N=256 -> 1 cyc/row).  Wanted values are the diagonal blocks
                # of the [4, 256] PSUM accumulator; gather them per-batch with
                # 4 tiny DMAs into a one-partition staging row, reduce at end.
                wb_col = smpool.tile([P, J], BF16, name=f"wb_col{b}", tag="wb_col")
                nc.vector.tensor_copy(wb_col, w_slab[:, b * J : (b + 1) * J])
                md_ps = pmd.tile([8, 8 * M], F32, name=f"md_ps{b}", tag="md")
                for t in range(J // 8):
                    nc.tensor.matmul(
                        md_ps,
                        wb_col[:, 8 * t : 8 * t + 8],
                        memb[:, 8 * t * M : (8 * t + 8) * M],
                        start=(t == 0),
                        stop=(t == J // 8 - 1),
                    )
                md_sb8 = smpool.tile([8, 8 * M], F32, name=f"md_sb8{b}", tag="md_sb8")
                nc.vector.tensor_copy(md_sb8, md_ps)
                for i in range(8):
                    nc.sync.dma_start(
                        out=md_gather[0:1, b * 8 * M + i * M : b * 8 * M + (i + 1) * M],
                        in_=md_sb8[i : i + 1, i * M : (i + 1) * M],
                    )
                nc.vector.tensor_reduce(
                    md_row[0:1, b * M : (b + 1) * M],
                    md_gather[0:1, b * 8 * M : (b + 1) * 8 * M].rearrange(
                        "a (i m) -> a m i", i=8
                    ),
                    AX.X,
                    ALU.add,
                )

            # ---------------- outputs ----------------
            nc.sync.dma_start(
                out=wout_d.rearrange("b (p j) -> p b j", p=P),
                in_=w_slab.rearrange("p (b j) -> p b j", b=B_LOC),
            )
            nc.sync.dma_start(out=md_d.rearrange("(a b) m -> a (b m)", a=1), in_=md_row)

    nc.finalize()
    return nc


def _get_nc():
    global _NC_CACHE
    if _NC_CACHE is None:
        _NC_CACHE = _build_nc()
    return _NC_CACHE


def kernel(**inputs):
    emb = np.ascontiguousarray(np.asarray(inputs["embeddings"], dtype=np.float32))
    wp = np.ascontiguousarray(np.asarray(inputs["w_prev"], dtype=np.float32))
    mem = np.ascontiguousarray(np.asarray(inputs["memory_weights"], dtype=np.float32))
    W = np.ascontiguousarray(np.asarray(inputs["W"], dtype=np.float32))
    bb = np.ascontiguousarray(np.asarray(inputs["b"], dtype=np.float32))

    nc = _get_nc()
    n_cores = 8
    in_maps = []
    for c in range(n_cores):
        sl = slice(c * B_LOC, (c + 1) * B_LOC)
        in_maps.append(
            {
                "embeddings": emb[sl],
                "w_prev": wp[sl],
                "memory_weights": mem[sl],
                "W": W,
                "b": bb,
            }
        )
    from concourse import bass_utils

    res = bass_utils.run_bass_kernel_spmd(nc, in_maps, list(range(n_cores)))
    md = np.concatenate([res.results[c]["memory_data"] for c in range(n_cores)], axis=0)
    w = np.concatenate([res.results[c]["w_out"] for c in range(n_cores)], axis=0)
    return md, w


# revision 30
# speedup vs baseline: 1.0359x; 1.0359x over previous
"""NTM read controller kernel for Trainium2 (8 NeuronCores, SPMD data-parallel over batch).

Problem shapes (hardcoded): B=64, E=512, N=8192, M=64, Dense out = M+6 = 70.
Sharding: batch 64 -> 8 cores x 8 batches. Each core is fully independent.

Per-core layout: n = 64*p + j  (partition p in [0,128), j in [0,64)).
memory_weights[b] loads as a [128, 4096] SBUF slab with 16 KiB contiguous per
partition (full-rate DMA).

Engine split per 4-batch group:
  phase A (per batch): DMA load; GpSimd k-multiply; DVE grouped reduces
     (dot, normsq); ACT square + bf16 cast.
  phase B (batched [128, 256] ops): the whole softmax/interp/shift/sharpen
     chain, with per-batch scalars broadcast along the free dim and
     partition sums / circular-shift boundaries done as single matmuls.
  phase C (per batch): bf16 weighted-read matmuls ([8, 512] PSUM), diagonal
     gather via small DMAs, grouped reduce to memory_data.
"""

import sys

for _p in ("/opt/trn_rl_repo", "/root/.axon_site/_ro/trn_rl_repo"):
    if _p not in sys.path:
        sys.path.insert(0, _p)

import numpy as np

import concourse.bass as bass
import concourse.bacc as bacc
import concourse.mybir as mybir
from concourse.tile import TileContext

F32 = mybir.dt.float32
BF16 = mybir.dt.bfloat16
AF = mybir.ActivationFunctionType
ALU = mybir.AluOpType
AX = mybir.AxisListType

B_LOC = 8      # batches per core
E = 512
M = 64
C_OUT = 70     # M + 6
P = 128        # partitions
J = 64         # n = 64*p + j
N = 8192
EPS = 1e-8
G = 2          # batch groups
BG = B_LOC // G  # batches per group (4)
W = BG * J       # chain op width (256)

_NC_CACHE = None


def _build_nc():
    nc = bacc.Bacc("TRN2", target_bir_lowering=False, debug=False, num_devices=8)

    emb_d = nc.dram_tensor("embeddings", [B_LOC, E], F32, kind="ExternalInput")
    wp_d = nc.dram_tensor("w_prev", [B_LOC, N], F32, kind="ExternalInput")
    mem_d = nc.dram_tensor("memory_weights", [B_LOC, N, M], F32, kind="ExternalInput")
    W_d = nc.dram_tensor("W", [E, C_OUT], F32, kind="ExternalInput")
    b_d = nc.dram_tensor("b", [C_OUT], F32, kind="ExternalInput")
    md_d = nc.dram_tensor("memory_data", [B_LOC, M], F32, kind="ExternalOutput")
    wout_d = nc.dram_tensor("w_out", [B_LOC, N], F32, kind="ExternalOutput")

    with TileContext(nc) as tc:
        with (
            tc.tile_pool(name="const", bufs=1) as cpool,
            tc.tile_pool(name="slab", bufs=2) as spool,
            tc.tile_pool(name="membp", bufs=6) as mbpool,
            tc.tile_pool(name="small", bufs=1) as smpool,
            tc.tile_pool(name="pscratch", bufs=2, space="PSUM") as ppool,
            tc.tile_pool(name="pmd", bufs=2, space="PSUM") as pmd,
        ):
            # ---------------- constants ----------------
            # Pre-load the one ACT table set covering ln/exp/square/copy so
            # the table-load pass never switches sets.
            _ltl = mybir.InstLoadActFuncSet(
                name=nc.get_next_instruction_name(), ins=[], outs=[]
            )
            _ltl.act_func_set_id = 6  # natural_log_exp_and_others
            nc.scalar.add_instruction(_ltl)

            ones128 = cpool.tile([P, P], F32)
            nc.vector.memset(ones128, 1.0)

            id8 = cpool.tile([8, 8], F32)
            nc.vector.memset(id8, 1.0)
            nc.gpsimd.affine_select(
                id8, id8, pattern=[[-1, 8]], compare_op=ALU.is_equal,
                fill=0.0, base=0, channel_multiplier=1,
            )

            # roll(+1): out[p] = in[(p-1) mod 128]
            sdn = cpool.tile([P, P], F32)
            nc.vector.memset(sdn, 1.0)
            nc.gpsimd.affine_select(
                sdn, sdn, pattern=[[-1, P]], compare_op=ALU.is_equal,
                fill=0.0, base=1, channel_multiplier=1,
            )
            sdn_wrap = cpool.tile([P, P], F32)
            nc.vector.memset(sdn_wrap, 1.0)
            nc.gpsimd.affine_select(
                sdn_wrap, sdn_wrap, pattern=[[-1, P]], compare_op=ALU.is_equal,
                fill=0.0, base=-(P - 1), channel_multiplier=1,
            )
            nc.vector.tensor_tensor(sdn, sdn, sdn_wrap, ALU.add)

            # roll(-1): out[p] = in[(p+1) mod 128]
            sup = cpool.tile([P, P], F32)
            nc.vector.memset(sup, 1.0)
            nc.gpsimd.affine_select(
                sup, sup, pattern=[[-1, P]], compare_op=ALU.is_equal,
                fill=0.0, base=-1, channel_multiplier=1,
            )
            nc.vector.memset(sup[0:1, P - 1 : P], 1.0)

            ones18 = cpool.tile([1, 8], F32)
            nc.vector.memset(ones18, 1.0)

            # ---------------- small inputs ----------------
            W4 = cpool.tile([P, 4 * C_OUT], F32)
            nc.sync.dma_start(
                out=W4.rearrange("p (c j) -> p c j", c=4),
                in_=W_d.rearrange("(c p) j -> p c j", p=P),
            )
            b_sb = cpool.tile([1, C_OUT], F32)
            nc.sync.dma_start(out=b_sb, in_=b_d.rearrange("(a j) -> a j", a=1))

            emb_sb = cpool.tile([B_LOC, E], F32)
            nc.sync.dma_start(out=emb_sb, in_=emb_d[:, :])

            wp_slab = cpool.tile([P, B_LOC * J], F32)
            nc.sync.dma_start(
                out=wp_slab.rearrange("p (b j) -> p b j", b=B_LOC),
                in_=wp_d.rearrange("b (p j) -> p b j", p=P),
            )

            w_slab = cpool.tile([P, B_LOC * J], F32)    # final w staging
            wb_slab = cpool.tile([P, B_LOC * J], BF16)  # bf16 w for read matmuls
            md_row = cpool.tile([1, B_LOC * M], F32)    # final memory_data staging
            md_gather = cpool.tile([1, B_LOC * 8 * M], F32)

            # ---------------- controller: addr = emb @ W + b ----------------
            eT_sb = cpool.tile([P, 32], F32)
            for c in range(4):
                eT_ps = ppool.tile([P, 8], F32, name=f"eT_ps{c}", tag="scratch")
                nc.tensor.matmul(
                    eT_ps, emb_sb[:, c * P : (c + 1) * P], id8, is_transpose=True
                )
                nc.vector.tensor_copy(eT_sb[:, c * 8 : (c + 1) * 8], eT_ps)

            addr_ps = ppool.tile([B_LOC, C_OUT], F32, tag="scratch")
            for c in range(4):
                nc.tensor.matmul(
                    addr_ps,
                    eT_sb[:, c * 8 : (c + 1) * 8],
                    W4.rearrange("p (c j) -> p c j", c=4)[:, c, :],
                    start=(c == 0),
                    stop=False,
                )
            nc.tensor.matmul(addr_ps, ones18, b_sb, start=False, stop=True)
            addr_sb = cpool.tile([B_LOC, C_OUT], F32)
            nc.vector.tensor_copy(addr_sb, addr_ps)

            # ---------------- controller nonlinearities ----------------
            # ctrl columns: 0=beta 1=g 2=1-g 3=y 4=k_norm 5..7=s
            ctrl = cpool.tile([B_LOC, 8], F32)
            sp_t = cpool.tile([B_LOC, 2], F32)
            nc.scalar.activation(sp_t[:, 0:1], addr_sb[:, 64:65], AF.Exp)
            nc.scalar.activation(sp_t[:, 1:2], addr_sb[:, 69:70], AF.Exp)
            nc.vector.tensor_scalar(sp_t, sp_t, 1.0, None, ALU.add)
            nc.scalar.activation(ctrl[:, 0:1], sp_t[:, 0:1], AF.Ln)   # beta
            nc.scalar.activation(ctrl[:, 3:4], sp_t[:, 1:2], AF.Ln)   # softplus(y)
            # sigmoid via exp: g = 1/(1+exp(-x))
            emg = cpool.tile([B_LOC, 1], F32)
            nc.scalar.activation(emg, addr_sb[:, 65:66], AF.Exp, scale=-1.0)
            nc.vector.tensor_scalar(emg, emg, 1.0, None, ALU.add)
            nc.vector.reciprocal(ctrl[:, 1:2], emg)
            nc.vector.tensor_scalar(
                ctrl[:, 2:3], ctrl[:, 1:2], -1.0, 1.0, ALU.mult, ALU.add
            )
            nc.vector.tensor_scalar(ctrl[:, 3:4], ctrl[:, 3:4], 1.0, None, ALU.add)

            ksq = cpool.tile([B_LOC, M], F32)
            knsq = cpool.tile([B_LOC, 1], F32)
            nc.scalar.activation(ksq, addr_sb[:, 0:M], AF.Square, accum_out=knsq)
            knl = cpool.tile([B_LOC, 1], F32)
            nc.scalar.activation(knl, knsq, AF.Ln)
            nc.scalar.activation(ctrl[:, 4:5], knl, AF.Exp, scale=0.5)

            sexp = cpool.tile([B_LOC, 3], F32)
            ssum = cpool.tile([B_LOC, 1], F32)
            nc.scalar.activation(sexp, addr_sb[:, 66:69], AF.Exp, accum_out=ssum)
            srec = cpool.tile([B_LOC, 1], F32)
            nc.vector.reciprocal(srec, ssum)
            nc.vector.tensor_scalar_mul(ctrl[:, 5:8], sexp, srec)

            # ---------------- broadcast controller scalars ----------------
            # scq[:, 8c:(c+1)8][p, b] = ctrl[b, c]  for every partition p.
            ctrlT_ps = ppool.tile([8, 8], F32, tag="scratch")
            nc.tensor.matmul(ctrlT_ps, ctrl, id8, is_transpose=True)
            ctrlT = cpool.tile([8, 8], F32)
            nc.vector.tensor_copy(ctrlT, ctrlT_ps)

            ones8p = cpool.tile([B_LOC, P], F32)
            nc.vector.memset(ones8p, 1.0)
            scq = cpool.tile([P, 8 * 8], F32)
            for c in range(8):
                selc = cpool.tile([B_LOC, P], F32, name=f"selc{c}", tag="sel", bufs=2)
                nc.gpsimd.affine_select(
                    selc, ones8p, pattern=[[0, P]], compare_op=ALU.is_equal,
                    fill=0.0, base=-c, channel_multiplier=1,
                )
                sc_ps = ppool.tile([P, 8], F32, name=f"sc_ps{c}", tag="scratch")
                nc.tensor.matmul(sc_ps, selc, ctrlT)
                nc.vector.tensor_copy(scq[:, 8 * c : 8 * (c + 1)], sc_ps)

            # per-batch k broadcast rows (for the elementwise k-multiply)
            kb_list = []
            for b in range(B_LOC):
                selb = cpool.tile([B_LOC, P], F32, name=f"selb{b}", tag="sel", bufs=2)
                nc.gpsimd.affine_select(
                    selb, ones8p, pattern=[[0, P]], compare_op=ALU.is_equal,
                    fill=0.0, base=-b, channel_multiplier=1,
                )
                kb_ps = ppool.tile([P, M], F32, name=f"kb_ps{b}", tag="scratch")
                nc.tensor.matmul(kb_ps, selb, addr_sb[:, 0:M])
                kb = cpool.tile([P, M], F32, name=f"kb{b}")
                nc.vector.tensor_copy(kb, kb_ps)
                kb_list.append(kb)

            def gview(c, g):
                """[128, BG, J] broadcast view of controller scalar c over group g."""
                return (
                    scq[:, 8 * c + BG * g : 8 * c + BG * (g + 1)]
                    .unsqueeze(2)
                    .broadcast_to([P, BG, J])
                )

            memb_list = [None] * B_LOC

            for g in range(G):
                bs = list(range(g * BG, (g + 1) * BG))
                gcol = slice(g * BG * J, (g + 1) * BG * J)

                dot_g = smpool.tile([P, W], F32, name=f"dot_g{g}", tag="dot_g")
                nsq_g = smpool.tile([P, W], F32, name=f"nsq_g{g}", tag="nsq_g")

                # ---------------- phase A ----------------
                for i, b in enumerate(bs):
                    mem_sb = spool.tile([P, J * M], F32, name=f"mem{b}", tag="mem")
                    nc.sync.dma_start(
                        out=mem_sb,
                        in_=mem_d[b].rearrange("(p j) m -> p (j m)", p=P),
                    )
                    mem3 = mem_sb.rearrange("p (j m) -> p j m", m=M)

                    prod = spool.tile([P, J * M], F32, name=f"prod{b}", tag="work")
                    kb3 = kb_list[b].unsqueeze(1).broadcast_to([P, J, M])
                    nc.gpsimd.tensor_tensor(
                        prod.rearrange("p (j m) -> p j m", m=M), mem3, kb3, ALU.mult
                    )
                    nc.vector.tensor_reduce(
                        dot_g[:, i * J : (i + 1) * J],
                        prod.rearrange("p (j m) -> p j m", m=M),
                        AX.X,
                        ALU.add,
                    )

                    sq = spool.tile([P, J * M], F32, name=f"sq{b}", tag="work")
                    nc.scalar.activation(sq, mem_sb, AF.Square)
                    nc.vector.tensor_reduce(
                        nsq_g[:, i * J : (i + 1) * J],
                        sq.rearrange("p (j m) -> p j m", m=M),
                        AX.X,
                        ALU.add,
                    )

                    memb = mbpool.tile([P, J * M], BF16, name=f"memb{b}", tag="memb")
                    nc.scalar.activation(memb, mem_sb, AF.Copy)
                    memb_list[b] = memb

                # ---------------- phase B (batched chain, width W=256) ------
                def t3(ap):
                    return ap.rearrange("p (b j) -> p b j", b=BG)

                lnn = smpool.tile([P, W], F32, name=f"lnn{g}", tag="lnn")
                nc.scalar.activation(lnn, nsq_g, AF.Ln)
                mnorm = smpool.tile([P, W], F32, name=f"mnorm{g}", tag="mnorm")
                nc.scalar.activation(mnorm, lnn, AF.Exp, scale=0.5)

                den = smpool.tile([P, W], F32, name=f"den{g}", tag="den")
                nc.vector.tensor_tensor(t3(den), t3(mnorm), gview(4, g), ALU.mult)
                nc.vector.tensor_scalar(den, den, EPS, None, ALU.add)
                rden = smpool.tile([P, W], F32, name=f"rden{g}", tag="rden")
                nc.vector.reciprocal(rden, den)
                # arg = beta * dot / den
                sim = smpool.tile([P, W], F32, name=f"sim{g}", tag="sim")
                nc.vector.tensor_tensor(sim, dot_g, rden, ALU.mult)
                arg = smpool.tile([P, W], F32, name=f"arg{g}", tag="arg")
                nc.vector.tensor_tensor(t3(arg), t3(sim), gview(0, g), ALU.mult)
                e_g = smpool.tile([P, W], F32, name=f"e_g{g}", tag="e_g")
                nc.scalar.activation(e_g, arg, AF.Exp)

                esum = smpool.tile([P, BG], F32, name=f"esum{g}", tag="esum")
                nc.vector.tensor_reduce(esum, t3(e_g), AX.X, ALU.add)
                z_ps = ppool.tile([P, BG], F32, name=f"z_ps{g}", tag="scratch")
                nc.tensor.matmul(z_ps, ones128, esum)
                zr = smpool.tile([P, BG], F32, name=f"zr{g}", tag="zr")
                nc.vector.reciprocal(zr, z_ps)
                gz = smpool.tile([P, BG], F32, name=f"gz{g}", tag="gz")
                nc.vector.tensor_tensor(
                    gz, zr, scq[:, 8 * 1 + BG * g : 8 * 1 + BG * (g + 1)], ALU.mult
                )

                # w_g = e*(g/Z) + (1-g)*w_prev
                ta = smpool.tile([P, W], F32, name=f"ta{g}", tag="ta")
                nc.vector.tensor_tensor(
                    t3(ta), t3(e_g), gz.unsqueeze(2).broadcast_to([P, BG, J]), ALU.mult
                )
                tb = smpool.tile([P, W], F32, name=f"tb{g}", tag="tb")
                nc.vector.tensor_tensor(t3(tb), t3(wp_slab[:, gcol]), gview(2, g), ALU.mult)
                wg = smpool.tile([P, W], F32, name=f"wg{g}", tag="wg")
                nc.vector.tensor_tensor(wg, ta, tb, ALU.add)

                # circular rolls along n = 64p + j (batched)
                r1 = smpool.tile([P, W], F32, name=f"r1{g}", tag="r1")
                nc.vector.tensor_copy(
                    r1.rearrange("p (b j) -> p b j", b=BG)[:, :, 1:J],
                    wg.rearrange("p (b j) -> p b j", b=BG)[:, :, 0 : J - 1],
                )
                c1_ps = ppool.tile([P, BG], F32, name=f"c1_ps{g}", tag="scratch")
                nc.tensor.matmul(
                    c1_ps, sdn, wg.rearrange("p (b j) -> p b j", b=BG)[:, :, J - 1]
                )
                nc.vector.tensor_copy(
                    r1.rearrange("p (b j) -> p b j", b=BG)[:, :, 0], c1_ps
                )

                rm1 = smpool.tile([P, W], F32, name=f"rm1{g}", tag="rm1")
                nc.vector.tensor_copy(
                    rm1.rearrange("p (b j) -> p b j", b=BG)[:, :, 0 : J - 1],
                    wg.rearrange("p (b j) -> p b j", b=BG)[:, :, 1:J],
                )
                cm_ps = ppool.tile([P, BG], F32, name=f"cm_ps{g}", tag="scratch")
                nc.tensor.matmul(
                    cm_ps, sup, wg.rearrange("p (b j) -> p b j", b=BG)[:, :, 0]
                )
                nc.vector.tensor_copy(
                    rm1.rearrange("p (b j) -> p b j", b=BG)[:, :, J - 1], cm_ps
                )

                # w_s = s0*r1 + s1*wg + s2*rm1
                a2 = smpool.tile([P, W], F32, name=f"a2{g}", tag="a2")
                nc.vector.tensor_tensor(t3(a2), t3(wg), gview(6, g), ALU.mult)
                tc1 = smpool.tile([P, W], F32, name=f"tc1{g}", tag="tc1")
                nc.vector.tensor_tensor(t3(tc1), t3(r1), gview(5, g), ALU.mult)
                t12 = smpool.tile([P, W], F32, name=f"t12{g}", tag="t12")
                nc.vector.tensor_tensor(t12, tc1, a2, ALU.add)
                tc2 = smpool.tile([P, W], F32, name=f"tc2{g}", tag="tc2")
                nc.vector.tensor_tensor(t3(tc2), t3(rm1), gview(7, g), ALU.mult)
                ws = smpool.tile([P, W], F32, name=f"ws{g}", tag="ws")
                nc.vector.tensor_tensor(ws, t12, tc2, ALU.add)

                # w = ws^y / (sum + EPS)
                lnw = smpool.tile([P, W], F32, name=f"lnw{g}", tag="lnw")
                nc.scalar.activation(lnw, ws, AF.Ln)
                arg2 = smpool.tile([P, W], F32, name=f"arg2{g}", tag="arg2")
                nc.vector.tensor_tensor(t3(arg2), t3(lnw), gview(3, g), ALU.mult)
                wpw = smpool.tile([P, W], F32, name=f"wpw{g}", tag="wpw")
                nc.scalar.activation(wpw, arg2, AF.Exp)

                wps = smpool.tile([P, BG], F32, name=f"wps{g}", tag="wps")
                nc.vector.tensor_reduce(wps, t3(wpw), AX.X, ALU.add)
                zp_ps = ppool.tile([P, BG], F32, name=f"zp_ps{g}", tag="scratch")
                nc.tensor.matmul(zp_ps, ones128, wps)
                zpe = smpool.tile([P, BG], F32, name=f"zpe{g}", tag="zpe")
                nc.vector.tensor_scalar(zpe, zp_ps, EPS, None, ALU.add)
                zpr = smpool.tile([P, BG], F32, name=f"zpr{g}", tag="zpr")
                nc.vector.reciprocal(zpr, zpe)
                nc.vector.tensor_tensor(
                    t3(w_slab[:, gcol]),
                    t3(wpw),
                    zpr.unsqueeze(2).broadcast_to([P, BG, J]),
                    ALU.mult,
                )
                nc.vector.tensor_copy(wb_slab[:, gcol], w_slab[:, gcol])

                # ---------------- phase C (reads) ----------------
                for b in bs:
                    memb = memb_list[b]
                    md_ps = pmd.tile([8, 8 * M], F32, name=f"md_ps{b}", tag="md")
                    for t in range(J // 8):
                        nc.tensor.matmul(
                            md_ps,
                            wb_slab[:, b * J + 8 * t : b * J + 8 * t + 8],
                            memb[:, 8 * t * M : (8 * t + 8) * M],
                            start=(t == 0),
                            stop=(t == J // 8 - 1),
                        )
                    md_sb8 = smpool.tile(
                        [8, 8 * M], F32, name=f"md_sb8{b}", tag="md_sb8", bufs=2
                    )
                    nc.vector.tensor_copy(md_sb8, md_ps)
                    for i in range(8):
                        nc.sync.dma_start(
                            out=md_gather[
                                0:1, b * 8 * M + i * M : b * 8 * M + (i + 1) * M
                            ],
                            in_=md_sb8[i : i + 1, i * M : (i + 1) * M],
                        )
                    nc.vector.tensor_reduce(
                        md_row[0:1, b * M : (b + 1) * M],
                        md_gather[0:1, b * 8 * M : (b + 1) * 8 * M].rearrange(
                            "a (i m) -> a m i", i=8
                        ),
                        AX.X,
                        ALU.add,
                    )

            # ---------------- outputs ----------------
            nc.sync.dma_start(
                out=wout_d.rearrange("b (p j) -> p b j", p=P),
                in_=w_slab.rearrange("p (b j) -> p b j", b=B_LOC),
            )
            nc.sync.dma_start(
                out=md_d.rearrange("(a b) m -> a (b m)", a=1), in_=md_row
            )

    nc.finalize()
    return nc


def _get_nc():
    global _NC_CACHE
    if _NC_CACHE is None:
        _NC_CACHE = _build_nc()
    return _NC_CACHE


def kernel(**inputs):
    emb = np.ascontiguousarray(np.asarray(inputs["embeddings"], dtype=np.float32))
    wp = np.ascontiguousarray(np.asarray(inputs["w_prev"], dtype=np.float32))
    mem = np.ascontiguousarray(np.asarray(inputs["memory_weights"], dtype=np.float32))
    Wm = np.ascontiguousarray(np.asarray(inputs["W"], dtype=np.float32))
    bb = np.ascontiguousarray(np.asarray(inputs["b"], dtype=np.float32))

    nc = _get_nc()
    n_cores = 8
    in_maps = []
    for c in range(n_cores):
        sl = slice(c * B_LOC, (c + 1) * B_LOC)
        in_maps.append(
            {
                "embeddings": emb[sl],
                "w_prev": wp[sl],
                "memory_weights": mem[sl],
                "W": Wm,
                "b": bb,
            }
        )
    from concourse import bass_utils

    res = bass_utils.run_bass_kernel_spmd(nc, in_maps, list(range(n_cores)))
    md = np.concatenate([res.results[c]["memory_data"] for c in range(n_cores)], axis=0)
    w = np.concatenate([res.results[c]["w_out"] for c in range(n_cores)], axis=0)
    return md, w


# revision 32
# speedup vs baseline: 1.1882x; 1.1470x over previous
"""NTM read controller kernel for Trainium2 (8 NeuronCores, SPMD data-parallel over batch).

Problem shapes (hardcoded): B=64, E=512, N=8192, M=64, Dense out = M+6 = 70.
Sharding: batch 64 -> 8 cores x 8 batches. Each core is fully independent.

Per-core layout: n = 64*p + j  (partition p in [0,128), j in [0,64)).
memory_weights[b] loads as a [128, 4096] SBUF slab with 16 KiB contiguous per
partition (full-rate DMA).

Engine split per 4-batch group:
  phase A (per batch): DMA load; GpSimd k-multiply; DVE grouped reduces
     (dot, normsq); ACT square + bf16 cast.
  phase B (batched [128, 256] ops): the whole softmax/interp/shift/sharpen
     chain, with per-batch scalars broadcast along the free dim and
     partition sums / circular-shift boundaries done as single matmuls.
  phase C (per batch): bf16 weighted-read matmuls ([8, 512] PSUM), diagonal
     gather via small DMAs, grouped reduce to memory_data.
"""

import sys

for _p in ("/opt/trn_rl_repo", "/root/.axon_site/_ro/trn_rl_repo"):
    if _p not in sys.path:
        sys.path.insert(0, _p)

import numpy as np

import concourse.bass as bass
import concourse.bacc as bacc
import concourse.mybir as mybir
from concourse.tile import TileContext

F32 = mybir.dt.float32
BF16 = mybir.dt.bfloat16
AF = mybir.ActivationFunctionType
ALU = mybir.AluOpType
AX = mybir.AxisListType

B_LOC = 8      # batches per core
E = 512
M = 64
C_OUT = 70     # M + 6
P = 128        # partitions
J = 64         # n = 64*p + j
N = 8192
EPS = 1e-8
G = 2          # batch groups
BG = B_LOC // G  # batches per group (4)
W = BG * J       # chain op width (256)

_NC_CACHE = None


def _build_nc():
    nc = bacc.Bacc("TRN2", target_bir_lowering=False, debug=False, num_devices=8)

    emb_d = nc.dram_tensor("embeddings", [B_LOC, E], F32, kind="ExternalInput")
    wp_d = nc.dram_tensor("w_prev", [B_LOC, N], F32, kind="ExternalInput")
    mem_d = nc.dram_tensor("memory_weights", [B_LOC, N, M], F32, kind="ExternalInput")
    W_d = nc.dram_tensor("W", [E, C_OUT], F32, kind="ExternalInput")
    b_d = nc.dram_tensor("b", [C_OUT], F32, kind="ExternalInput")
    md_d = nc.dram_tensor("memory_data", [B_LOC, M], F32, kind="ExternalOutput")
    wout_d = nc.dram_tensor("w_out", [B_LOC, N], F32, kind="ExternalOutput")

    with TileContext(nc) as tc:
        with (
            tc.tile_pool(name="const", bufs=1) as cpool,
            tc.tile_pool(name="slab", bufs=2) as spool,
            tc.tile_pool(name="membp", bufs=6) as mbpool,
            tc.tile_pool(name="small", bufs=1) as smpool,
            tc.tile_pool(name="pscratch", bufs=2, space="PSUM") as ppool,
            tc.tile_pool(name="pmd", bufs=2, space="PSUM") as pmd,
        ):
            # ---------------- constants ----------------
            # Pre-load the one ACT table set covering ln/exp/square/copy so
            # the table-load pass never switches sets.
            _ltl = mybir.InstLoadActFuncSet(
                name=nc.get_next_instruction_name(), ins=[], outs=[]
            )
            _ltl.act_func_set_id = 6  # natural_log_exp_and_others
            nc.scalar.add_instruction(_ltl)

            ones128 = cpool.tile([P, P], F32)
            nc.vector.memset(ones128, 1.0)

            id8 = cpool.tile([8, 8], F32)
            nc.vector.memset(id8, 1.0)
            nc.gpsimd.affine_select(
                id8, id8, pattern=[[-1, 8]], compare_op=ALU.is_equal,
                fill=0.0, base=0, channel_multiplier=1,
            )

            # roll(+1): out[p] = in[(p-1) mod 128]
            sdn = cpool.tile([P, P], F32)
            nc.vector.memset(sdn, 1.0)
            nc.gpsimd.affine_select(
                sdn, sdn, pattern=[[-1, P]], compare_op=ALU.is_equal,
                fill=0.0, base=1, channel_multiplier=1,
            )
            sdn_wrap = cpool.tile([P, P], F32)
            nc.vector.memset(sdn_wrap, 1.0)
            nc.gpsimd.affine_select(
                sdn_wrap, sdn_wrap, pattern=[[-1, P]], compare_op=ALU.is_equal,
                fill=0.0, base=-(P - 1), channel_multiplier=1,
            )
            nc.vector.tensor_tensor(sdn, sdn, sdn_wrap, ALU.add)

            # roll(-1): out[p] = in[(p+1) mod 128]
            sup = cpool.tile([P, P], F32)
            nc.vector.memset(sup, 1.0)
            nc.gpsimd.affine_select(
                sup, sup, pattern=[[-1, P]], compare_op=ALU.is_equal,
                fill=0.0, base=-1, channel_multiplier=1,
            )
            nc.vector.memset(sup[0:1, P - 1 : P], 1.0)

            ones18 = cpool.tile([1, 8], F32)
            nc.vector.memset(ones18, 1.0)

            # ---------------- small inputs ----------------
            W4 = cpool.tile([P, 4 * C_OUT], F32)
            nc.sync.dma_start(
                out=W4.rearrange("p (c j) -> p c j", c=4),
                in_=W_d.rearrange("(c p) j -> p c j", p=P),
            )
            b_sb = cpool.tile([1, C_OUT], F32)
            nc.sync.dma_start(out=b_sb, in_=b_d.rearrange("(a j) -> a j", a=1))

            emb_sb = cpool.tile([B_LOC, E], F32)
            nc.sync.dma_start(out=emb_sb, in_=emb_d[:, :])

            wp_slab = cpool.tile([P, B_LOC * J], F32)
            nc.sync.dma_start(
                out=wp_slab.rearrange("p (b j) -> p b j", b=B_LOC),
                in_=wp_d.rearrange("b (p j) -> p b j", p=P),
            )

            w_slab = cpool.tile([P, B_LOC * J], F32)    # final w staging
            wb_slab = cpool.tile([P, B_LOC * J], BF16)  # bf16 w for read matmuls
            md_row = cpool.tile([1, B_LOC * M], F32)    # final memory_data staging
            md_gather = cpool.tile([1, B_LOC * 8 * M], F32)

            # ---------------- controller: addr = emb @ W + b ----------------
            eT_sb = cpool.tile([P, 32], F32)
            for c in range(4):
                eT_ps = ppool.tile([P, 8], F32, name=f"eT_ps{c}", tag="scratch")
                nc.tensor.matmul(
                    eT_ps, emb_sb[:, c * P : (c + 1) * P], id8, is_transpose=True
                )
                nc.vector.tensor_copy(eT_sb[:, c * 8 : (c + 1) * 8], eT_ps)

            addr_ps = ppool.tile([B_LOC, C_OUT], F32, tag="scratch")
            for c in range(4):
                nc.tensor.matmul(
                    addr_ps,
                    eT_sb[:, c * 8 : (c + 1) * 8],
                    W4.rearrange("p (c j) -> p c j", c=4)[:, c, :],
                    start=(c == 0),
                    stop=False,
                )
            nc.tensor.matmul(addr_ps, ones18, b_sb, start=False, stop=True)
            addr_sb = cpool.tile([B_LOC, C_OUT], F32)
            nc.vector.tensor_copy(addr_sb, addr_ps)

            # ---------------- controller nonlinearities ----------------
            # ctrl columns: 0=beta 1=g 2=1-g 3=y 4=k_norm 5..7=s
            ctrl = cpool.tile([B_LOC, 8], F32)
            sp_t = cpool.tile([B_LOC, 2], F32)
            nc.scalar.activation(sp_t[:, 0:1], addr_sb[:, 64:65], AF.Exp)
            nc.scalar.activation(sp_t[:, 1:2], addr_sb[:, 69:70], AF.Exp)
            nc.vector.tensor_scalar(sp_t, sp_t, 1.0, None, ALU.add)
            nc.scalar.activation(ctrl[:, 0:1], sp_t[:, 0:1], AF.Ln)   # beta
            nc.scalar.activation(ctrl[:, 3:4], sp_t[:, 1:2], AF.Ln)   # softplus(y)
            # sigmoid via exp: g = 1/(1+exp(-x))
            emg = cpool.tile([B_LOC, 1], F32)
            nc.scalar.activation(emg, addr_sb[:, 65:66], AF.Exp, scale=-1.0)
            nc.vector.tensor_scalar(emg, emg, 1.0, None, ALU.add)
            nc.vector.reciprocal(ctrl[:, 1:2], emg)
            nc.vector.tensor_scalar(
                ctrl[:, 2:3], ctrl[:, 1:2], -1.0, 1.0, ALU.mult, ALU.add
            )
            nc.vector.tensor_scalar(ctrl[:, 3:4], ctrl[:, 3:4], 1.0, None, ALU.add)

            ksq = cpool.tile([B_LOC, M], F32)
            knsq = cpool.tile([B_LOC, 1], F32)
            nc.scalar.activation(ksq, addr_sb[:, 0:M], AF.Square, accum_out=knsq)
            knl = cpool.tile([B_LOC, 1], F32)
            nc.scalar.activation(knl, knsq, AF.Ln)
            nc.scalar.activation(ctrl[:, 4:5], knl, AF.Exp, scale=0.5)

            sexp = cpool.tile([B_LOC, 3], F32)
            ssum = cpool.tile([B_LOC, 1], F32)
            nc.scalar.activation(sexp, addr_sb[:, 66:69], AF.Exp, accum_out=ssum)
            srec = cpool.tile([B_LOC, 1], F32)
            nc.vector.reciprocal(srec, ssum)
            nc.vector.tensor_scalar_mul(ctrl[:, 5:8], sexp, srec)

            # ---------------- broadcast controller scalars ----------------
            # scq[:, 8c:(c+1)8][p, b] = ctrl[b, c]  for every partition p.
            ctrlT_ps = ppool.tile([8, 8], F32, tag="scratch")
            nc.tensor.matmul(ctrlT_ps, ctrl, id8, is_transpose=True)
            ctrlT = cpool.tile([8, 8], F32)
            nc.vector.tensor_copy(ctrlT, ctrlT_ps)

            ones8p = cpool.tile([B_LOC, P], F32)
            nc.vector.memset(ones8p, 1.0)
            scq = cpool.tile([P, 8 * 8], F32)
            for c in range(8):
                selc = cpool.tile([B_LOC, P], F32, name=f"selc{c}", tag="sel", bufs=2)
                nc.gpsimd.affine_select(
                    selc, ones8p, pattern=[[0, P]], compare_op=ALU.is_equal,
                    fill=0.0, base=-c, channel_multiplier=1,
                )
                sc_ps = ppool.tile([P, 8], F32, name=f"sc_ps{c}", tag="scratch")
                nc.tensor.matmul(sc_ps, selc, ctrlT)
                nc.vector.tensor_copy(scq[:, 8 * c : 8 * (c + 1)], sc_ps)

            # per-batch k broadcast rows (for the elementwise k-multiply)
            kb_list = []
            for b in range(B_LOC):
                selb = cpool.tile([B_LOC, P], F32, name=f"selb{b}", tag="sel", bufs=2)
                nc.gpsimd.affine_select(
                    selb, ones8p, pattern=[[0, P]], compare_op=ALU.is_equal,
                    fill=0.0, base=-b, channel_multiplier=1,
                )
                kb_ps = ppool.tile([P, M], F32, name=f"kb_ps{b}", tag="scratch")
                nc.tensor.matmul(kb_ps, selb, addr_sb[:, 0:M])
                kbb = cpool.tile([P, M], BF16, name=f"kbb{b}")
                nc.vector.tensor_copy(kbb, kb_ps)
                kb_list.append(kbb)

            def gview(c, g):
                """[128, BG, J] broadcast view of controller scalar c over group g."""
                return (
                    scq[:, 8 * c + BG * g : 8 * c + BG * (g + 1)]
                    .unsqueeze(2)
                    .broadcast_to([P, BG, J])
                )

            memb_list = [None] * B_LOC
            dot_tiles = {}
            nsq_tiles = {}

            def phase_a(g):
                bs = list(range(g * BG, (g + 1) * BG))
                dot_g = smpool.tile([P, W], F32, name=f"dot_g{g}", tag=f"dot_g{g}")
                nsq_g = smpool.tile([P, W], F32, name=f"nsq_g{g}", tag=f"nsq_g{g}")
                dot_tiles[g] = dot_g
                nsq_tiles[g] = nsq_g
                for i, b in enumerate(bs):
                    mem_sb = spool.tile([P, J * M], F32, name=f"mem{b}", tag="mem")
                    nc.sync.dma_start(
                        out=mem_sb,
                        in_=mem_d[b].rearrange("(p j) m -> p (j m)", p=P),
                    )

                    memb = mbpool.tile([P, J * M], BF16, name=f"memb{b}", tag="memb")
                    nc.scalar.activation(memb, mem_sb, AF.Copy)
                    memb_list[b] = memb
                    memb3 = memb.rearrange("p (j m) -> p j m", m=M)

                    # dot product in bf16 on DVE (2x mode), reduce in fp32
                    prod = spool.tile([P, J * M], BF16, name=f"prod{b}", tag="prodb")
                    kb3 = kb_list[b].unsqueeze(1).broadcast_to([P, J, M])
                    nc.vector.tensor_tensor(
                        prod.rearrange("p (j m) -> p j m", m=M), memb3, kb3, ALU.mult
                    )
                    nc.vector.tensor_reduce(
                        dot_g[:, i * J : (i + 1) * J],
                        prod.rearrange("p (j m) -> p j m", m=M),
                        AX.X,
                        ALU.add,
                    )

                    sq = spool.tile([P, J * M], F32, name=f"sq{b}", tag="work")
                    nc.scalar.activation(sq, mem_sb, AF.Square)
                    nc.vector.tensor_reduce(
                        nsq_g[:, i * J : (i + 1) * J],
                        sq.rearrange("p (j m) -> p j m", m=M),
                        AX.X,
                        ALU.add,
                    )

            def phase_b(g):
                gcol = slice(g * BG * J, (g + 1) * BG * J)
                dot_g = dot_tiles[g]
                nsq_g = nsq_tiles[g]

                def t3(ap):
                    return ap.rearrange("p (b j) -> p b j", b=BG)

                lnn = smpool.tile([P, W], F32, name=f"lnn{g}", tag="lnn")
                nc.scalar.activation(lnn, nsq_g, AF.Ln)
                mnorm = smpool.tile([P, W], F32, name=f"mnorm{g}", tag="mnorm")
                nc.scalar.activation(mnorm, lnn, AF.Exp, scale=0.5)

                den = smpool.tile([P, W], F32, name=f"den{g}", tag="den")
                nc.vector.tensor_tensor(t3(den), t3(mnorm), gview(4, g), ALU.mult)
                nc.vector.tensor_scalar(den, den, EPS, None, ALU.add)
                rden = smpool.tile([P, W], F32, name=f"rden{g}", tag="rden")
                nc.vector.reciprocal(rden, den)
                # arg = beta * dot / den
                sim = smpool.tile([P, W], F32, name=f"sim{g}", tag="sim")
                nc.vector.tensor_tensor(sim, dot_g, rden, ALU.mult)
                arg = smpool.tile([P, W], F32, name=f"arg{g}", tag="arg")
                nc.vector.tensor_tensor(t3(arg), t3(sim), gview(0, g), ALU.mult)
                e_g = smpool.tile([P, W], F32, name=f"e_g{g}", tag="e_g")
                nc.scalar.activation(e_g, arg, AF.Exp)

                esum = smpool.tile([P, BG], F32, name=f"esum{g}", tag="esum")
                nc.vector.tensor_reduce(esum, t3(e_g), AX.X, ALU.add)
                z_ps = ppool.tile([P, BG], F32, name=f"z_ps{g}", tag="scratch")
                nc.tensor.matmul(z_ps, ones128, esum)
                zr = smpool.tile([P, BG], F32, name=f"zr{g}", tag="zr")
                nc.vector.reciprocal(zr, z_ps)
                gz = smpool.tile([P, BG], F32, name=f"gz{g}", tag="gz")
                nc.vector.tensor_tensor(
                    gz, zr, scq[:, 8 * 1 + BG * g : 8 * 1 + BG * (g + 1)], ALU.mult
                )

                # w_g = e*(g/Z) + (1-g)*w_prev
                ta = smpool.tile([P, W], F32, name=f"ta{g}", tag="ta")
                nc.vector.tensor_tensor(
                    t3(ta), t3(e_g), gz.unsqueeze(2).broadcast_to([P, BG, J]), ALU.mult
                )
                tb = smpool.tile([P, W], F32, name=f"tb{g}", tag="tb")
                nc.vector.tensor_tensor(t3(tb), t3(wp_slab[:, gcol]), gview(2, g), ALU.mult)
                wg = smpool.tile([P, W], F32, name=f"wg{g}", tag="wg")
                nc.vector.tensor_tensor(wg, ta, tb, ALU.add)

                # circular rolls along n = 64p + j (batched)
                r1 = smpool.tile([P, W], F32, name=f"r1{g}", tag="r1")
                nc.vector.tensor_copy(
                    r1.rearrange("p (b j) -> p b j", b=BG)[:, :, 1:J],
                    wg.rearrange("p (b j) -> p b j", b=BG)[:, :, 0 : J - 1],
                )
                c1_ps = ppool.tile([P, BG], F32, name=f"c1_ps{g}", tag="scratch")
                nc.tensor.matmul(
                    c1_ps, sdn, wg.rearrange("p (b j) -> p b j", b=BG)[:, :, J - 1]
                )
                nc.vector.tensor_copy(
                    r1.rearrange("p (b j) -> p b j", b=BG)[:, :, 0], c1_ps
                )

                rm1 = smpool.tile([P, W], F32, name=f"rm1{g}", tag="rm1")
                nc.vector.tensor_copy(
                    rm1.rearrange("p (b j) -> p b j", b=BG)[:, :, 0 : J - 1],
                    wg.rearrange("p (b j) -> p b j", b=BG)[:, :, 1:J],
                )
                cm_ps = ppool.tile([P, BG], F32, name=f"cm_ps{g}", tag="scratch")
                nc.tensor.matmul(
                    cm_ps, sup, wg.rearrange("p (b j) -> p b j", b=BG)[:, :, 0]
                )
                nc.vector.tensor_copy(
                    rm1.rearrange("p (b j) -> p b j", b=BG)[:, :, J - 1], cm_ps
                )

                # w_s = s0*r1 + s1*wg + s2*rm1
                a2 = smpool.tile([P, W], F32, name=f"a2{g}", tag="a2")
                nc.vector.tensor_tensor(t3(a2), t3(wg), gview(6, g), ALU.mult)
                tc1 = smpool.tile([P, W], F32, name=f"tc1{g}", tag="tc1")
                nc.vector.tensor_tensor(t3(tc1), t3(r1), gview(5, g), ALU.mult)
                t12 = smpool.tile([P, W], F32, name=f"t12{g}", tag="t12")
                nc.vector.tensor_tensor(t12, tc1, a2, ALU.add)
                tc2 = smpool.tile([P, W], F32, name=f"tc2{g}", tag="tc2")
                nc.vector.tensor_tensor(t3(tc2), t3(rm1), gview(7, g), ALU.mult)
                ws = smpool.tile([P, W], F32, name=f"ws{g}", tag="ws")
                nc.vector.tensor_tensor(ws, t12, tc2, ALU.add)

                # w = ws^y / (sum + EPS)
                lnw = smpool.tile([P, W], F32, name=f"lnw{g}", tag="lnw")
                nc.scalar.activation(lnw, ws, AF.Ln)
                arg2 = smpool.tile([P, W], F32, name=f"arg2{g}", tag="arg2")
                nc.vector.tensor_tensor(t3(arg2), t3(lnw), gview(3, g), ALU.mult)
                wpw = smpool.tile([P, W], F32, name=f"wpw{g}", tag="wpw")
                nc.scalar.activation(wpw, arg2, AF.Exp)

                wps = smpool.tile([P, BG], F32, name=f"wps{g}", tag="wps")
                nc.vector.tensor_reduce(wps, t3(wpw), AX.X, ALU.add)
                zp_ps = ppool.tile([P, BG], F32, name=f"zp_ps{g}", tag="scratch")
                nc.tensor.matmul(zp_ps, ones128, wps)
                zpe = smpool.tile([P, BG], F32, name=f"zpe{g}", tag="zpe")
                nc.vector.tensor_scalar(zpe, zp_ps, EPS, None, ALU.add)
                zpr = smpool.tile([P, BG], F32, name=f"zpr{g}", tag="zpr")
                nc.vector.reciprocal(zpr, zpe)
                nc.vector.tensor_tensor(
                    t3(w_slab[:, gcol]),
                    t3(wpw),
                    zpr.unsqueeze(2).broadcast_to([P, BG, J]),
                    ALU.mult,
                )
                nc.vector.tensor_copy(wb_slab[:, gcol], w_slab[:, gcol])

            def phase_c(g):
                bs = list(range(g * BG, (g + 1) * BG))
                for b in bs:
                    memb = memb_list[b]
                    md_ps = pmd.tile([8, 8 * M], F32, name=f"md_ps{b}", tag="md")
                    for t in range(J // 8):
                        nc.tensor.matmul(
                            md_ps,
                            wb_slab[:, b * J + 8 * t : b * J + 8 * t + 8],
                            memb[:, 8 * t * M : (8 * t + 8) * M],
                            start=(t == 0),
                            stop=(t == J // 8 - 1),
                        )
                    md_sb8 = smpool.tile(
                        [8, 8 * M], F32, name=f"md_sb8{b}", tag="md_sb8", bufs=2
                    )
                    nc.vector.tensor_copy(md_sb8, md_ps)
                    for i in range(8):
                        nc.sync.dma_start(
                            out=md_gather[
                                0:1, b * 8 * M + i * M : b * 8 * M + (i + 1) * M
                            ],
                            in_=md_sb8[i : i + 1, i * M : (i + 1) * M],
                        )
                    nc.vector.tensor_reduce(
                        md_row[0:1, b * M : (b + 1) * M],
                        md_gather[0:1, b * 8 * M : (b + 1) * 8 * M].rearrange(
                            "a (i m) -> a m i", i=8
                        ),
                        AX.X,
                        ALU.add,
                    )

            # emission order keeps the DMA FIFO free of gather-behind-load
            # stalls and overlaps group phases
            phase_a(0)
            phase_b(0)
            phase_a(1)
            phase_c(0)
            phase_b(1)
            phase_c(1)

            # ---------------- outputs ----------------
            nc.sync.dma_start(
                out=wout_d.rearrange("b (p j) -> p b j", p=P),
                in_=w_slab.rearrange("p (b j) -> p b j", b=B_LOC),
            )
            nc.sync.dma_start(
                out=md_d.rearrange("(a b) m -> a (b m)", a=1), in_=md_row
            )

    nc.finalize()
    return nc


def _get_nc():
    global _NC_CACHE
    if _NC_CACHE is None:
        _NC_CACHE = _build_nc()
    return _NC_CACHE


def kernel(**inputs):
    emb = np.ascontiguousarray(np.asarray(inputs["embeddings"], dtype=np.float32))
    wp = np.ascontiguousarray(np.asarray(inputs["w_prev"], dtype=np.float32))
    mem = np.ascontiguousarray(np.asarray(inputs["memory_weights"], dtype=np.float32))
    Wm = np.ascontiguousarray(np.asarray(inputs["W"], dtype=np.float32))
    bb = np.ascontiguousarray(np.asarray(inputs["b"], dtype=np.float32))

    nc = _get_nc()
    n_cores = 8
    in_maps = []
    for c in range(n_cores):
        sl = slice(c * B_LOC, (c + 1) * B_LOC)
        in_maps.append(
            {
                "embeddings": emb[sl],
                "w_prev": wp[sl],
                "memory_weights": mem[sl],
                "W": Wm,
                "b": bb,
            }
        )
    from concourse import bass_utils

    res = bass_utils.run_bass_kernel_spmd(nc, in_maps, list(range(n_cores)))
    md = np.concatenate([res.results[c]["memory_data"] for c in range(n_cores)], axis=0)
    w = np.concatenate([res.results[c]["w_out"] for c in range(n_cores)], axis=0)
    return md, w
